# revision 1
# baseline (speedup 1.0000x reference)
"""MoE routing kernel for 8 Trainium2 NeuronCores (Bass/Tile, SPMD).

Strategy (expert-parallel, matching the sharding hint):
  - Host computes the gate (softmax + top-2) and dispatches tokens: each of
    the 8 cores owns 2 of the 16 routed experts and receives only the tokens
    routed to its experts (gathered + transposed + zero-padded to a common
    capacity). This is the "all-to-all token dispatch on the topk indices".
  - The output layer (ow) is linear and commutes with the weighted combine,
    so it is folded into each expert's second matmul on the host
    (w2ot = w2[e].T @ ow.T), shrinking stage-2 work by W/OUT = 4x.
  - The shared expert is sharded over its intermediate dim (2048/8=256 rows
    per core); every core computes a partial for all 2048 tokens, also with
    ow folded in.  Bias terms that commute with the output layer
    (b2, sb2, ob) are applied analytically on the host.
  - Device matmuls run in float32r (full-rate fp32 on the PE array).
  - Host combines: scatter-add of combine-weight-scaled routed partials +
    shared partials + analytic bias terms.
"""
import sys

if "/opt/trn_rl_repo" not in sys.path:
    sys.path.insert(0, "/opt/trn_rl_repo")

import numpy as np
import concourse.bass as bass
import concourse.tile as tile
from concourse import mybir
from concourse.bass_utils import run_bass_kernel_spmd

B = 2048
W = 512
E = 16
TOPK = 2
INTER = 1024
SH = 2048
OUT = 128
NCORES = 8
EPC = E // NCORES          # experts per core = 2
SHS = SH // NCORES         # shared-expert inter slice per core = 256
KW = W // 128              # k-tiles over W = 4
MI = INTER // 128          # m-tiles over INTER = 8
MS = SHS // 128            # m-tiles over shared slice = 2
F32 = mybir.dt.float32
F32R = mybir.dt.float32r
F16 = mybir.dt.float16
DT = F16                   # device datapath dtype for matmul operands
NPDT = np.float16

# set by test.py to collect a profile; results stashed in LAST_RESULTS
TRACE = False
TRACE_KW = {}
LAST_RESULTS = None


def _legalize_waits(nc):
    """This container's walrus accepts at most 1 sync wait per instruction
    (2 for EventSemaphore).  Hoist excess waits emitted by the Tile
    scheduler into standalone EventSemaphore instructions."""
    for fn in nc.m.functions:
        for blk in fn.blocks:
            out = []
            changed = False
            for inst in blk.instructions:
                si = getattr(inst, "sync_info", None)
                waits = list(si.on_wait) if si is not None and si.on_wait else []
                cap = 2 if isinstance(inst, mybir.InstEventSemaphore) else 1
                if len(waits) > cap:
                    extra, keep = waits[:-cap], waits[-cap:]
                    for i in range(0, len(extra), 2):
                        out.append(mybir.InstEventSemaphore(
                            name=nc.get_next_instruction_name(),
                            engine=inst.engine,
                            ins=[], outs=[],
                            sync_info=mybir.SyncInfo(
                                on_wait=list(extra[i:i + 2]), on_update=[]),
                        ))
                    si.on_wait = keep
                    changed = True
                out.append(inst)
            if changed:
                blk.instructions = out


def _token_chunks(cap):
    """Split [0, cap) into chunks of <=512 (all multiples of 128)."""
    chunks = []
    off = 0
    while off < cap:
        sz = min(512, cap - off)
        chunks.append((off, sz))
        off += sz
    return chunks


def _build_nc(cap, loop_n=None, legalize=True, mode="full"):
    """Build the SPMD Bass program for per-expert token capacity `cap`
    (multiple of 128).  loop_n wraps the body in a hardware For_i loop
    (used only for timing measurements)."""
    nc = bass.Bass("TRN2", target_bir_lowering=False, debug=False)

    def din(name, f, dt=DT):
        return nc.dram_tensor(name, [128, f], dt, kind="ExternalInput").ap()

    xt = din("xt", KW * B)                   # x.T packed: col block k = x.T[128k:128k+128, :]
    xg = din("xg", EPC * KW * cap)           # gathered tokens per expert, packed like xt
    w1t = din("w1t", EPC * KW * INTER)       # per expert: w1[e].T packed k-blocks
    w3t = din("w3t", EPC * KW * INTER)
    w2ot = din("w2ot", EPC * MI * OUT)       # per expert: (w2[e].T @ ow.T) packed k-blocks
    sw1t = din("sw1t", KW * SHS)             # shared slice: sw1[s].T packed
    sw3t = din("sw3t", KW * SHS)
    sw2ot = din("sw2ot", MS * OUT)           # (sw2[:, s].T @ ow.T) packed
    bias = din("bias", EPC * 2 * MI + 2 * MS, F32)  # b1/b3 per expert (8 cols each), sb1/sb3 (2 cols each)

    yr = nc.dram_tensor("yr", [128, EPC * cap], F32, kind="ExternalOutput").ap()
    zt = nc.dram_tensor("zt", [128, B], F32, kind="ExternalOutput").ap()

    LR = mybir.ActivationFunctionType.Lrelu
    IDT = mybir.ActivationFunctionType.Identity

    with tile.TileContext(nc) as tc:
        import contextlib
        with tc.tile_pool(name="wts", bufs=1) as wts, \
             tc.tile_pool(name="work", bufs=2) as work, \
             tc.tile_pool(name="hts", bufs=1) as hts, \
             tc.tile_pool(name="outs", bufs=2) as outs, \
             tc.tile_pool(name="ps", bufs=2, space="PSUM") as ps, \
             contextlib.ExitStack() as _loopstack:
            _loop_entered = [False]

            def _enter_loop():
                if loop_n is not None and not _loop_entered[0]:
                    _loopstack.enter_context(tc.For_i(
                        0, loop_n, 1,
                        hint_engines=(mybir.EngineType.PE,
                                      mybir.EngineType.Activation,
                                      mybir.EngineType.DVE,
                                      mybir.EngineType.SP)))
                    _loop_entered[0] = True
            if mode != "compute":
                _enter_loop()

            def emit_body():
              # ---- tiny bias + shared-expert inputs go on otherwise-idle DMA
              # queues (Pool/ACT) so the SP queue is dedicated to expert weights.
              bias_t = wts.tile([128, bias.shape[1]], F32, tag="bias")
              nc.scalar.dma_start(bias_t[:], bias[:])
              sw1_ts, sw3_ts = [], []
              for k in range(KW):
                  t = wts.tile([128, SHS], DT, tag=f"sw1k{k}")
                  nc.scalar.dma_start(t[:], sw1t[:, k * SHS:(k + 1) * SHS])
                  sw1_ts.append(t)
                  t = wts.tile([128, SHS], DT, tag=f"sw3k{k}")
                  nc.scalar.dma_start(t[:], sw3t[:, k * SHS:(k + 1) * SHS])
                  sw3_ts.append(t)
              sw2_t = wts.tile([128, MS * OUT], DT, tag="sw2")
              nc.scalar.dma_start(sw2_t[:], sw2ot[:])
              xt_ts = []
              for k in range(KW):
                  t = wts.tile([128, B], DT, tag=f"xtk{k}")
                  nc.scalar.dma_start(t[:], xt[:, k * B:(k + 1) * B])
                  xt_ts.append(t)

              def b_ap(col):  # [128,1] per-partition bias column
                  return bias_t[:, col:col + 1]

              chunks = _token_chunks(cap)

              def expert_steps(e):
                  """Generator: step 0 = weight DMAs, then one step per
                  (token-chunk, m-tile) with stage-2 interleaved."""
                  w1_ts, w3_ts, xg_ts = [], [], []
                  for k in range(KW):
                      t = work.tile([128, INTER], DT, tag=f"w1k{k}", bufs=3)
                      nc.sync.dma_start(t[:], w1t[:, (e * KW + k) * INTER:(e * KW + k + 1) * INTER])
                      w1_ts.append(t)
                      t = work.tile([128, INTER], DT, tag=f"w3k{k}", bufs=3)
                      nc.sync.dma_start(t[:], w3t[:, (e * KW + k) * INTER:(e * KW + k + 1) * INTER])
                      w3_ts.append(t)
                      t = work.tile([128, cap], DT, tag=f"xgk{k}")
                      nc.sync.dma_start(t[:], xg[:, (e * KW + k) * cap:(e * KW + k + 1) * cap])
                      xg_ts.append(t)
                  w2_t = work.tile([128, MI * OUT], DT, tag="w2")
                  nc.sync.dma_start(w2_t[:], w2ot[:, e * MI * OUT:(e + 1) * MI * OUT])
                  yield

                  LAG = 2   # stage-2 MMs trail stage 1 so the in-order PE
                            # stream never stalls on the ACT->DVE h chain
                  for (c0, csz) in chunks:
                      py = ps.tile([128, csz], F32, tag="py")
                      hts_pend = []
                      for m in range(MI):
                          p1 = ps.tile([128, csz], F32, tag="p1", bufs=3)
                          p3 = ps.tile([128, csz], F32, tag="p3", bufs=3)
                          for k in range(KW):
                              lhs1 = w1_ts[k][:, m * 128:(m + 1) * 128]
                              rhs = xg_ts[k][:, c0:c0 + csz]
                              nc.tensor.matmul(p1[:], lhs1, rhs, start=(k == 0), stop=(k == KW - 1))
                          for k in range(KW):
                              lhs3 = w3_ts[k][:, m * 128:(m + 1) * 128]
                              rhs = xg_ts[k][:, c0:c0 + csz]
                              nc.tensor.matmul(p3[:], lhs3, rhs, start=(k == 0), stop=(k == KW - 1))
                          a = work.tile([128, csz], DT, tag="act_a")
                          nc.scalar.activation(a[:], p1[:], LR, bias=b_ap(e * 2 * MI + m), alpha=0.01)
                          t3 = work.tile([128, csz], DT, tag="act_b")
                          nc.vector.tensor_scalar_add(t3[:], p3[:], b_ap(e * 2 * MI + MI + m))
                          ht = hts.tile([128, csz], DT, tag=f"ht{m}", bufs=3)
                          nc.vector.tensor_mul(ht[:], a[:], t3[:])
                          hts_pend.append((m, ht))
                          if len(hts_pend) > LAG:
                              md, htd = hts_pend.pop(0)
                              lhs = w2_t[:, md * OUT:(md + 1) * OUT]
                              nc.tensor.matmul(py[:], lhs, htd[:], start=(md == 0), stop=(md == MI - 1))
                          if not (m == MI - 1 and (c0, csz) == chunks[-1]):
                              yield
                      for md, htd in hts_pend:
                          lhs = w2_t[:, md * OUT:(md + 1) * OUT]
                          nc.tensor.matmul(py[:], lhs, htd[:], start=(md == 0), stop=(md == MI - 1))
                      yo = outs.tile([128, csz], F32, tag="yo")
                      nc.vector.tensor_copy(yo[:], py[:])
                      nc.sync.dma_start(yr[:, e * cap + c0: e * cap + c0 + csz], yo[:])
                  yield

              def shared_steps():
                  """Generator: one step per (token-group, m-tile); the pz
                  stage-2 accumulation trails by one group to avoid PE stalls."""
                  pend = []   # (pz, zo-flush closure) per group

                  def flush(pzg, gc0, gcsz, hs_list):
                      for m, hs in hs_list:
                          lhs = sw2_t[:, m * OUT:(m + 1) * OUT]
                          nc.tensor.matmul(pzg[:], lhs, hs[:], start=(m == 0), stop=(m == MS - 1))
                      zo = outs.tile([128, gcsz], F32, tag="zo")
                      nc.vector.tensor_copy(zo[:], pzg[:])
                      nc.sync.dma_start(zt[:, gc0:gc0 + gcsz], zo[:])

                  for (c0, csz) in _token_chunks(B):
                      pz = ps.tile([128, csz], F32, tag="py")
                      hs_list = []
                      for m in range(MS):
                          p1 = ps.tile([128, csz], F32, tag="p1", bufs=3)
                          p3 = ps.tile([128, csz], F32, tag="p3", bufs=3)
                          for k in range(KW):
                              lhs1 = sw1_ts[k][:, m * 128:(m + 1) * 128]
                              rhs = xt_ts[k][:, c0:c0 + csz]
                              nc.tensor.matmul(p1[:], lhs1, rhs, start=(k == 0), stop=(k == KW - 1))
                          for k in range(KW):
                              lhs3 = sw3_ts[k][:, m * 128:(m + 1) * 128]
                              rhs = xt_ts[k][:, c0:c0 + csz]
                              nc.tensor.matmul(p3[:], lhs3, rhs, start=(k == 0), stop=(k == KW - 1))
                          a = work.tile([128, csz], DT, tag="act_a")
                          nc.scalar.activation(a[:], p1[:], LR, bias=b_ap(EPC * 2 * MI + m), alpha=0.01)
                          t3 = work.tile([128, csz], DT, tag="act_b")
                          nc.vector.tensor_scalar_add(t3[:], p3[:], b_ap(EPC * 2 * MI + MS + m))
                          hs = hts.tile([128, csz], DT, tag=f"hs{m}", bufs=3)
                          nc.vector.tensor_mul(hs[:], a[:], t3[:])
                          hs_list.append((m, hs))
                          yield
                      pend.append((pz, c0, csz, hs_list))
                      if len(pend) > 1:
                          flush(*pend.pop(0))
                  for args_ in pend:
                      flush(*args_)
                  yield

              # fine-grained interleave of the expert stream (SP DMA queue) and
              # the shared-expert stream (ACT DMA queue): emission order sets
              # scheduler priority, so merging at m-tile granularity lets each
              # stream fill PE stalls caused by the other's weight DMAs.
              import os as _os
              if mode == "dma":
                  for g in [expert_steps(e) for e in range(EPC)]:
                      next(g)   # DMA prologue only
              elif mode == "experts":
                  for g in [expert_steps(e) for e in range(EPC)]:
                      for _ in g:
                          pass
              elif mode == "shared":
                  for _ in shared_steps():
                      pass
              else:
                  pattern = _os.environ.get(
                      "K_PATTERN", "E ESSE EES EES EE ESSE EES EES EE SSSS")
                  elist = [expert_steps(e) for e in range(EPC)]
                  if mode == "compute":
                      for g in elist:
                          next(g)          # emit weight DMAs outside the loop
                      _enter_loop()        # loop wraps compute only
                  sgen = shared_steps()
                  ei = 0
                  for ch in pattern:
                      if ch == " ":
                          continue
                      if ch == "E":
                          if elist[ei] is None:
                              continue
                          try:
                              next(elist[ei])
                          except StopIteration:
                              elist[ei] = None
                              ei = min(ei + 1, EPC - 1)
                      else:
                          try:
                              next(sgen)
                          except StopIteration:
                              pass
                  for g in elist + [sgen]:
                      if g is None:
                          continue
                      for _ in g:
                          pass


            if mode.startswith("u"):
                for _r in range(int(mode[1:])):
                    emit_body()
            else:
                emit_body()

    if legalize:
        _legalize_waits(nc)
    return nc


_NC_CACHE = {}


def _pack_kblocks(mat):
    """[Ktot, F] -> [128, (Ktot/128)*F] with col block k = mat[128k:128(k+1), :]."""
    ktot, f = mat.shape
    assert ktot % 128 == 0
    return np.ascontiguousarray(
        mat.reshape(ktot // 128, 128, f).transpose(1, 0, 2).reshape(128, -1))


def prepare(x, task_id, gate_w, w1, b1, w2, b2, w3, b3,
            sw1, sb1, sw2, sb2, sw3, sb3, ow, ob):
    """Host-side routing + packing.  Returns everything needed to launch the
    device program and combine its partial outputs."""
    x = np.asarray(x, np.float32)
    f32 = lambda a: np.asarray(a, np.float32)
    gate_w, w1, b1, w2, b2, w3, b3 = map(f32, (gate_w, w1, b1, w2, b2, w3, b3))
    sw1, sb1, sw2, sb2, sw3, sb3, ow, ob = map(f32, (sw1, sb1, sw2, sb2, sw3, sb3, ow, ob))

    # ---- host gate: softmax + top-2 (the routing decision) ----
    logits = x @ gate_w.T
    logits -= logits.max(axis=1, keepdims=True)
    ex = np.exp(logits)
    scores = ex / ex.sum(axis=1, keepdims=True)            # [B, E] fp32
    order = np.argsort(-scores, axis=1, kind="stable")[:, :TOPK]   # [B, 2]

    tok_lists = []
    for e in range(E):
        sel = np.nonzero((order == e).any(axis=1))[0]
        tok_lists.append(sel)
    max_cnt = max(len(t) for t in tok_lists)
    cap = max(128, -(-max_cnt // 16) * 16)

    if cap not in _NC_CACHE:
        _NC_CACHE[cap] = _build_nc(cap)
    nc = _NC_CACHE[cap]

    # ---- pack per-core inputs (device datapath dtype) ----
    xt_p = _pack_kblocks(x.T.copy()).astype(NPDT)          # [128, KW*B]
    in_maps = []
    for c in range(NCORES):
        exps = [c * EPC + j for j in range(EPC)]
        xg_blocks, w1_bl, w3_bl, w2_bl = [], [], [], []
        bias_cols = []
        for e in exps:
            toks = tok_lists[e]
            xge = np.zeros((W, cap), np.float32)
            xge[:, :len(toks)] = x[toks].T
            xg_blocks.append(_pack_kblocks(xge).astype(NPDT))
            w1_bl.append(_pack_kblocks(w1[e].T.copy()).astype(NPDT))
            w3_bl.append(_pack_kblocks(w3[e].T.copy()).astype(NPDT))
            w2_bl.append(_pack_kblocks(w2[e].T @ ow.T).astype(NPDT))
        for e in exps:
            bias_cols.append(b1[e].reshape(MI, 128).T)     # [128, MI]
            bias_cols.append(b3[e].reshape(MI, 128).T)
        s = slice(c * SHS, (c + 1) * SHS)
        bias_cols.append(sb1[s].reshape(MS, 128).T)
        bias_cols.append(sb3[s].reshape(MS, 128).T)
        in_maps.append({
            "xt": xt_p,
            "xg": np.concatenate(xg_blocks, axis=1),
            "w1t": np.concatenate(w1_bl, axis=1),
            "w3t": np.concatenate(w3_bl, axis=1),
            "w2ot": np.concatenate(w2_bl, axis=1),
            "sw1t": _pack_kblocks(sw1[s].T.copy()).astype(NPDT),
            "sw3t": _pack_kblocks(sw3[s].T.copy()).astype(NPDT),
            "sw2ot": _pack_kblocks(sw2[:, s].T @ ow.T).astype(NPDT),
            "bias": np.ascontiguousarray(np.concatenate(bias_cols, axis=1)),
        })

    # dense combine weights [B, E] (zero except the top-2 experts per token)
    combine_w = np.zeros((B, E), np.float32)
    rows = np.arange(B)
    combine_w[rows[:, None], order] = np.take_along_axis(scores, order, axis=1)
    # analytic bias terms: sum_e combine[:,e] * (b2[e] @ ow.T)  +  sb2 @ ow.T + ob
    base = combine_w @ (b2 @ ow.T) + sb2 @ ow.T + ob

    return dict(nc=nc, cap=cap, in_maps=in_maps, tok_lists=tok_lists,
                combine_w=combine_w, base=base)


def combine(p, results):
    """Combine per-core device partials into the full [B, OUT] output."""
    cap, tok_lists, combine_w = p["cap"], p["tok_lists"], p["combine_w"]
    out = p["base"].astype(np.float32).copy()
    for c in range(NCORES):
        r = results[c]
        out += r["zt"].astype(np.float32).T
        for j in range(EPC):
            e = c * EPC + j
            toks = tok_lists[e]
            yre = r["yr"][:, j * cap: j * cap + len(toks)].astype(np.float32)  # [OUT, cnt]
            out[toks] += combine_w[toks, e][:, None] * yre.T
    return out


def kernel(x, task_id, gate_w, w1, b1, w2, b2, w3, b3,
           sw1, sb1, sw2, sb2, sw3, sb3, ow, ob):
    global LAST_RESULTS
    p = prepare(x, task_id, gate_w, w1, b1, w2, b2, w3, b3,
                sw1, sb1, sw2, sb2, sw3, sb3, ow, ob)
    res = run_bass_kernel_spmd(
        p["nc"], p["in_maps"], core_ids=list(range(NCORES)),
        trace=TRACE, **TRACE_KW)
    LAST_RESULTS = res
    return combine(p, res.results)



# revision 5
# speedup vs baseline: 1.1861x; 1.1861x over previous
"""MoE routing kernel for 8 Trainium2 NeuronCores (Bass/Tile, SPMD).

Strategy (expert-parallel, per the sharding hint):
  - Host computes the gate (softmax + top-2) and dispatches tokens: each of
    the 8 cores owns 2 of the 16 routed experts and receives only the tokens
    routed to its experts.  Experts are sorted by token count and paired
    (rank i with rank 15-i) so per-slot capacities (cap0 = max big-slot
    count, cap1 = max small-slot count) carry minimal padding.
  - The output layer (ow) commutes with the weighted combine, so it is
    folded into each expert's second matmul on the host (w2[e].T @ ow.T),
    shrinking stage-2 work 4x.  Bias terms that commute (b2, sb2, ob) are
    applied analytically on the host.
  - The shared expert is sharded over its intermediate dim (2048/8 = 256
    rows per core); every core computes a partial over all 2048 tokens.
  - Device-side scheduling: weights are packed in consumption order and
    DMAed in m-tile-group granularity on the two HWDGE queues (SP = expert
    weights, ACT = activations/shared) so the in-order PE stream starts
    ~3us in and pipelines with the weight stream.  A short dummy-matmul
    warmup flips the PE HAM clock gate to 2.4 GHz during the DMA ramp.
    Element-wise work is spread over ACT (lrelu+bias), DVE (bias add) and
    GpSimd (multiply).  Outputs stream out incrementally in fp16 on the
    SWDGE queue.
  - Host combines: scatter-add of combine-weight-scaled routed partials +
    shared partials + analytic bias terms.
"""
import sys

if "/opt/trn_rl_repo" not in sys.path:
    sys.path.insert(0, "/opt/trn_rl_repo")

import os
import numpy as np
import concourse.bass as bass
import concourse.tile as tile
from concourse import mybir
from concourse.bass_utils import run_bass_kernel_spmd

B = 2048
W = 512
E = 16
TOPK = 2
INTER = 1024
SH = 2048
OUT = 128
NCORES = 8
EPC = E // NCORES          # experts per core = 2
SHS = SH // NCORES         # shared-expert inter slice per core = 256
KW = W // 128              # k-tiles over W = 4
MI = INTER // 128          # m-tiles over INTER = 8
MS = SHS // 128            # m-tiles over shared slice = 2
GCOLS = 2 * KW * 256 + 2 * OUT   # columns per expert m-pair weight group
F32 = mybir.dt.float32
F16 = mybir.dt.float16
DT = F16                   # device datapath dtype for matmul operands
NPDT = np.float16

# set by test.py to collect a profile; results stashed in LAST_RESULTS
TRACE = False
TRACE_KW = {}
LAST_RESULTS = None


def _legalize_waits(nc):
    """This container's walrus accepts at most 1 sync wait per instruction
    (2 for EventSemaphore).  Hoist excess waits emitted by the Tile
    scheduler into standalone EventSemaphore instructions."""
    for fn in nc.m.functions:
        for blk in fn.blocks:
            out = []
            changed = False
            for inst in blk.instructions:
                si = getattr(inst, "sync_info", None)
                waits = list(si.on_wait) if si is not None and si.on_wait else []
                cap = 2 if isinstance(inst, mybir.InstEventSemaphore) else 1
                if len(waits) > cap:
                    extra, keep = waits[:-cap], waits[-cap:]
                    for i in range(0, len(extra), 2):
                        out.append(mybir.InstEventSemaphore(
                            name=nc.get_next_instruction_name(),
                            engine=inst.engine,
                            ins=[], outs=[],
                            sync_info=mybir.SyncInfo(
                                on_wait=list(extra[i:i + 2]), on_update=[]),
                        ))
                    si.on_wait = keep
                    changed = True
                out.append(inst)
            if changed:
                blk.instructions = out
    return nc


def _build_nc(cap0, cap1, legalize=True):
    """SPMD Bass program for per-slot token capacities (cap0, cap1)."""
    nc = bass.Bass("TRN2", target_bir_lowering=False, debug=False)
    caps = (cap0, cap1)

    def din(name, f, dt=DT):
        return nc.dram_tensor(name, [128, f], dt, kind="ExternalInput").ap()

    xga = din("xga", KW * cap0)     # gathered tokens, slot A, k-major
    xgb = din("xgb", KW * cap1)
    wga = din("wga", 4 * GCOLS)     # slot A weights: 4 m-pair groups
    wgb = din("wgb", 4 * GCOLS)
    swp = din("swp", GCOLS)         # shared weights: sw1 | sw3 | sw2ot
    xt = din("xt", KW * B)          # x.T in 4 chunk-major blocks of [128, KW*512]
    bias = din("bias", 4 * MI + 2 * MS, F32)

    yr = nc.dram_tensor("yr", [128, cap0 + cap1], DT, kind="ExternalOutput").ap()
    zt = nc.dram_tensor("zt", [128, B], DT, kind="ExternalOutput").ap()

    LR = mybir.ActivationFunctionType.Lrelu

    with tile.TileContext(nc) as tc:
        with tc.tile_pool(name="wts", bufs=1) as wts, \
             tc.tile_pool(name="work", bufs=3) as work, \
             tc.tile_pool(name="hts", bufs=1) as hts, \
             tc.tile_pool(name="outs", bufs=2) as outs, \
             tc.tile_pool(name="ps", bufs=2, space="PSUM") as ps:

            # ---- PE warmup tile (zeroed by Pool engine; no DMA dependency)
            warm = wts.tile([128, 512], DT, tag="warm")
            nc.gpsimd.memset(warm[:], 0.0)

            # ---- input DMAs, consumption-ordered on the two HWDGE queues
            bias_t = wts.tile([128, bias.shape[1]], F32, tag="bias")
            xga_t = wts.tile([128, KW * cap0], DT, tag="xga")
            xgb_t = wts.tile([128, KW * cap1], DT, tag="xgb")
            swp_t = wts.tile([128, GCOLS], DT, tag="swp")
            xt_ts = [wts.tile([128, KW * 512], DT, tag=f"xt{c}", name=f"xt{c}")
                     for c in range(4)]
            wga_ts = [wts.tile([128, GCOLS], DT, tag=f"wga{g}", name=f"wga{g}")
                      for g in range(4)]
            wgb_ts = [wts.tile([128, GCOLS], DT, tag=f"wgb{g}", name=f"wgb{g}")
                      for g in range(4)]

            # ACT queue: activations + shared stream
            nc.scalar.dma_start(bias_t[:], bias[:])
            nc.scalar.dma_start(xga_t[:], xga[:])
            nc.scalar.dma_start(swp_t[:], swp[:])
            nc.scalar.dma_start(xt_ts[0][:], xt[:, 0:KW * 512])
            nc.scalar.dma_start(xgb_t[:], xgb[:])
            for c in range(1, 4):
                nc.scalar.dma_start(xt_ts[c][:], xt[:, c * KW * 512:(c + 1) * KW * 512])
            # SP queue: expert weights (first group split per m-tile so
            # compute can start as soon as ~0.3 MB lands)
            h0 = GCOLS // 2
            nc.sync.dma_start(wga_ts[0][:, :h0], wga[:, :h0])
            nc.sync.dma_start(wga_ts[0][:, h0:], wga[:, h0:GCOLS])
            for g in range(1, 4):
                nc.sync.dma_start(wga_ts[g][:], wga[:, g * GCOLS:(g + 1) * GCOLS])
            for g in range(4):
                nc.sync.dma_start(wgb_ts[g][:], wgb[:, g * GCOLS:(g + 1) * GCOLS])

            # ---- PE warmup: ~9 cold N=512 matmuls ~= 3.5us, flips HAM to 8/8
            pw = ps.tile([128, 512], F32, tag="warm", bufs=1)
            for i in range(9):
                nc.tensor.matmul(pw[:], warm[:, 0:128], warm[:],
                                 start=True, stop=True)

            def b_ap(col):  # [128,1] per-partition bias column
                return bias_t[:, col:col + 1]

            LAG = 2

            def expert_slot(slot):
                """Generator: one step per m-tile, stage-2 trails by LAG."""
                cap = caps[slot]
                xg_t = (xga_t, xgb_t)[slot]
                wg_ts = (wga_ts, wgb_ts)[slot]
                boff = slot * 2 * MI
                py = ps.tile([128, cap], F32, tag="py", bufs=1)
                pend = []

                def stage2(m, ht):
                    g, mloc = divmod(m, 2)
                    lhs = wg_ts[g][:, 2 * KW * 256 + mloc * OUT:
                                   2 * KW * 256 + (mloc + 1) * OUT]
                    nc.tensor.matmul(py[:], lhs, ht[:],
                                     start=(m == 0), stop=(m == MI - 1))

                for m in range(MI):
                    g, mloc = divmod(m, 2)
                    wt = wg_ts[g]
                    p1 = ps.tile([128, cap], F32, tag="p1", bufs=2)
                    p3 = ps.tile([128, cap], F32, tag="p3", bufs=2)
                    for k in range(KW):
                        lhs = wt[:, (mloc * KW + k) * 128:(mloc * KW + k + 1) * 128]
                        rhs = xg_t[:, k * cap:(k + 1) * cap]
                        nc.tensor.matmul(p1[:], lhs, rhs, start=(k == 0), stop=(k == KW - 1))
                    for k in range(KW):
                        lhs = wt[:, (KW * 256 + (mloc * KW + k) * 128):
                                 (KW * 256 + (mloc * KW + k + 1) * 128)]
                        rhs = xg_t[:, k * cap:(k + 1) * cap]
                        nc.tensor.matmul(p3[:], lhs, rhs, start=(k == 0), stop=(k == KW - 1))
                    a = work.tile([128, cap], DT, tag="act_a")
                    nc.scalar.activation(a[:], p1[:], LR, bias=b_ap(boff + m), alpha=0.01)
                    t3 = work.tile([128, cap], DT, tag="act_b")
                    nc.vector.tensor_scalar_add(t3[:], p3[:], b_ap(boff + MI + m))
                    ht = hts.tile([128, cap], DT, tag="ht", bufs=LAG + 3)
                    nc.gpsimd.tensor_mul(ht[:], a[:], t3[:])
                    pend.append((m, ht))
                    if len(pend) > LAG:
                        stage2(*pend.pop(0))
                    if m != MI - 1:
                        yield
                for args in pend:
                    stage2(*args)
                yo = outs.tile([128, cap], DT, tag="yo")
                nc.vector.tensor_copy(yo[:], py[:])
                off = 0 if slot == 0 else cap0
                nc.gpsimd.dma_start(yr[:, off:off + cap], yo[:])
                yield

            def shared_steps():
                """Generator: one step per (chunk, m-tile); stage-2 flush of
                chunk c happens during chunk c+1 (full-chunk lag)."""
                pend = []

                def flush(pz, c, hs_list):
                    for m, hs in hs_list:
                        lhs = swp_t[:, 2 * KW * 256 + m * OUT:
                                    2 * KW * 256 + (m + 1) * OUT]
                        nc.tensor.matmul(pz[:], lhs, hs[:], start=(m == 0), stop=(m == MS - 1))
                    zo = outs.tile([128, 512], DT, tag="zo")
                    nc.vector.tensor_copy(zo[:], pz[:])
                    nc.gpsimd.dma_start(zt[:, c * 512:(c + 1) * 512], zo[:])

                for c in range(4):
                    pz = ps.tile([128, 512], F32, tag="pz", bufs=1)
                    hs_list = []
                    for m in range(MS):
                        p1 = ps.tile([128, 512], F32, tag="p1", bufs=2)
                        p3 = ps.tile([128, 512], F32, tag="p3", bufs=2)
                        for k in range(KW):
                            lhs = swp_t[:, (m * KW + k) * 128:(m * KW + k + 1) * 128]
                            rhs = xt_ts[c][:, k * 512:(k + 1) * 512]
                            nc.tensor.matmul(p1[:], lhs, rhs, start=(k == 0), stop=(k == KW - 1))
                        for k in range(KW):
                            lhs = swp_t[:, (KW * 256 + (m * KW + k) * 128):
                                        (KW * 256 + (m * KW + k + 1) * 128)]
                            rhs = xt_ts[c][:, k * 512:(k + 1) * 512]
                            nc.tensor.matmul(p3[:], lhs, rhs, start=(k == 0), stop=(k == KW - 1))
                        a = work.tile([128, 512], DT, tag="act_a")
                        nc.scalar.activation(a[:], p1[:], LR,
                                             bias=b_ap(4 * MI + m), alpha=0.01)
                        t3 = work.tile([128, 512], DT, tag="act_b")
                        nc.vector.tensor_scalar_add(t3[:], p3[:], b_ap(4 * MI + MS + m))
                        hs = hts.tile([128, 512], DT, tag="hs", bufs=MS + 2)
                        nc.gpsimd.tensor_mul(hs[:], a[:], t3[:])
                        hs_list.append((m, hs))
                        if not (c == 3 and m == MS - 1):
                            yield
                    pend.append((pz, c, hs_list))
                    if len(pend) > 1:
                        flush(*pend.pop(0))
                for args in pend:
                    flush(*args)
                yield

            # ---- interleave: A/B = expert slot m-tile, S = shared m-tile.
            # Default tuned to the DMA arrival schedule (SP ~0.58MB/group,
            # ACT xga+swp+xt chunks); override via K_PATTERN for experiments.
            pattern = os.environ.get(
                "K_PATTERN",
                "AAAA SS AA SS AA SS BB SS BBBBBB")
            gens = {"A": expert_slot(0), "B": expert_slot(1), "S": shared_steps()}
            for ch in pattern:
                if ch == " ":
                    continue
                g = gens.get(ch)
                if g is None:
                    continue
                try:
                    next(g)
                except StopIteration:
                    gens[ch] = None
            for g in gens.values():
                if g is None:
                    continue
                for _ in g:
                    pass

    if legalize:
        _legalize_waits(nc)
    return nc


_NC_CACHE = {}


def _kblocks(mat, nk):
    """[nk*128, F] -> [128, nk*F], col block k = mat[128k:128(k+1), :]."""
    f = mat.shape[1]
    return mat.reshape(nk, 128, f).transpose(1, 0, 2).reshape(128, -1)


def _pack_expert(w1e, w3e, w2oe):
    """Consumption-ordered weight pack for one expert: 4 groups of
    [w1 m-pair k-blocks | w3 m-pair k-blocks | w2ot m-pair], GCOLS each."""
    # w1e/w3e: [INTER, W] -> per (m, k) blocks [128, 128]
    w1b = w1e.T.reshape(KW, 128, MI, 128)   # [k, kr, m, mc]
    w3b = w3e.T.reshape(KW, 128, MI, 128)
    w2b = w2oe.reshape(MI, 128, OUT)        # [m, mr, OUT]
    groups = []
    for g in range(4):
        cols = []
        for mloc in range(2):
            m = 2 * g + mloc
            for k in range(KW):
                cols.append(w1b[k, :, m, :])
        for mloc in range(2):
            m = 2 * g + mloc
            for k in range(KW):
                cols.append(w3b[k, :, m, :])
        for mloc in range(2):
            cols.append(w2b[2 * g + mloc])
        groups.append(np.concatenate(cols, axis=1))
    return np.concatenate(groups, axis=1).astype(NPDT)


def prepare(x, task_id, gate_w, w1, b1, w2, b2, w3, b3,
            sw1, sb1, sw2, sb2, sw3, sb3, ow, ob):
    """Host-side routing + packing."""
    x = np.asarray(x, np.float32)
    f32 = lambda a: np.asarray(a, np.float32)
    gate_w, w1, b1, w2, b2, w3, b3 = map(f32, (gate_w, w1, b1, w2, b2, w3, b3))
    sw1, sb1, sw2, sb2, sw3, sb3, ow, ob = map(f32, (sw1, sb1, sw2, sb2, sw3, sb3, ow, ob))

    # ---- host gate: softmax + top-2 ----
    logits = x @ gate_w.T
    logits -= logits.max(axis=1, keepdims=True)
    ex = np.exp(logits)
    scores = ex / ex.sum(axis=1, keepdims=True)
    order = np.argsort(-scores, axis=1, kind="stable")[:, :TOPK]

    tok_lists = [np.nonzero((order == e).any(axis=1))[0] for e in range(E)]
    counts = np.array([len(t) for t in tok_lists])
    rank = np.argsort(-counts, kind="stable")
    slotA = [int(rank[i]) for i in range(NCORES)]          # big experts
    slotB = [int(rank[E - 1 - i]) for i in range(NCORES)]  # small experts
    r16 = lambda n: max(64, -(-n // 16) * 16)
    cap0 = r16(max(counts[e] for e in slotA))
    cap1 = r16(max(counts[e] for e in slotB))

    key = (cap0, cap1)
    if key not in _NC_CACHE:
        _NC_CACHE[key] = _build_nc(cap0, cap1)
    nc = _NC_CACHE[key]

    # ---- shared-expert packing (same for every core except slice) ----
    xtp = x.T.reshape(KW, 128, 4, 512).transpose(1, 2, 0, 3).reshape(128, -1)
    xtp = np.ascontiguousarray(xtp).astype(NPDT)   # chunk-major, k inside

    in_maps = []
    for c in range(NCORES):
        eA, eB = slotA[c], slotB[c]
        s = slice(c * SHS, (c + 1) * SHS)

        def gather(e, cap):
            toks = tok_lists[e]
            xg = np.zeros((W, cap), np.float32)
            xg[:, :len(toks)] = x[toks].T
            return _kblocks(xg, KW).astype(NPDT)

        # shared pack mirrors the expert group layout (sw1 | sw3 | sw2ot)
        sw1b = sw1[s].T.reshape(KW, 128, MS, 128)
        sw3b = sw3[s].T.reshape(KW, 128, MS, 128)
        sw2o = (sw2[:, s].T @ ow.T).reshape(MS, 128, OUT)
        cols = []
        for m in range(MS):
            for k in range(KW):
                cols.append(sw1b[k, :, m, :])
        for m in range(MS):
            for k in range(KW):
                cols.append(sw3b[k, :, m, :])
        for m in range(MS):
            cols.append(sw2o[m])
        swp = np.concatenate(cols, axis=1).astype(NPDT)

        bias_cols = [b1[eA].reshape(MI, 128).T, b3[eA].reshape(MI, 128).T,
                     b1[eB].reshape(MI, 128).T, b3[eB].reshape(MI, 128).T,
                     sb1[s].reshape(MS, 128).T, sb3[s].reshape(MS, 128).T]

        in_maps.append({
            "xga": gather(eA, cap0),
            "xgb": gather(eB, cap1),
            "wga": _pack_expert(w1[eA], w3[eA], w2[eA].T @ ow.T),
            "wgb": _pack_expert(w1[eB], w3[eB], w2[eB].T @ ow.T),
            "swp": swp,
            "xt": xtp,
            "bias": np.ascontiguousarray(np.concatenate(bias_cols, axis=1)),
        })

    combine_w = np.zeros((B, E), np.float32)
    rows = np.arange(B)
    combine_w[rows[:, None], order] = np.take_along_axis(scores, order, axis=1)
    base = combine_w @ (b2 @ ow.T) + sb2 @ ow.T + ob

    return dict(nc=nc, in_maps=in_maps, cap0=cap0, cap1=cap1,
                slotA=slotA, slotB=slotB, tok_lists=tok_lists,
                combine_w=combine_w, base=base)


def combine(p, results):
    """Combine per-core device partials into the full [B, OUT] output."""
    cap0, tok_lists, combine_w = p["cap0"], p["tok_lists"], p["combine_w"]
    out = p["base"].astype(np.float32).copy()
    for c in range(NCORES):
        r = results[c]
        out += r["zt"].astype(np.float32).T
        for slot, e in ((0, p["slotA"][c]), (1, p["slotB"][c])):
            toks = tok_lists[e]
            off = 0 if slot == 0 else cap0
            yre = r["yr"][:, off:off + len(toks)].astype(np.float32)
            out[toks] += combine_w[toks, e][:, None] * yre.T
    return out


def kernel(x, task_id, gate_w, w1, b1, w2, b2, w3, b3,
           sw1, sb1, sw2, sb2, sw3, sb3, ow, ob):
    global LAST_RESULTS
    p = prepare(x, task_id, gate_w, w1, b1, w2, b2, w3, b3,
                sw1, sb1, sw2, sb2, sw3, sb3, ow, ob)
    res = run_bass_kernel_spmd(
        p["nc"], p["in_maps"], core_ids=list(range(NCORES)),
        trace=TRACE, **TRACE_KW)
    LAST_RESULTS = res
    return combine(p, res.results)


# revision 13
# speedup vs baseline: 1.1929x; 1.0057x over previous
"""MoE routing kernel for 8 Trainium2 NeuronCores (Bass/Tile, SPMD).

Strategy (expert-parallel, per the sharding hint):
  - Host computes the gate (softmax + top-2) and dispatches tokens: each of
    the 8 cores owns 2 of the 16 routed experts and receives only the tokens
    routed to its experts.  Experts are sorted by token count and paired
    (rank i with rank 15-i) so per-slot capacities (cap0 = max big-slot
    count, cap1 = max small-slot count) carry minimal padding.
  - The output layer (ow) commutes with the weighted combine, so it is
    folded into each expert's second matmul on the host (w2[e].T @ ow.T),
    shrinking stage-2 work 4x.  Bias terms that commute (b2, sb2, ob) are
    applied analytically on the host.
  - The shared expert is sharded over its intermediate dim (2048/8 = 256
    rows per core); every core computes a partial over all 2048 tokens.
  - Device-side scheduling: weights are packed in consumption order and
    DMAed in m-tile-group granularity on the two HWDGE queues (SP = expert
    weights, ACT = activations/shared) so the in-order PE stream starts
    ~3us in and pipelines with the weight stream.  A short dummy-matmul
    warmup flips the PE HAM clock gate to 2.4 GHz during the DMA ramp.
    Element-wise work is spread over ACT (lrelu+bias), DVE (bias add) and
    GpSimd (multiply).  Outputs stream out incrementally in fp16 on the
    SWDGE queue.
  - Host combines: scatter-add of combine-weight-scaled routed partials +
    shared partials + analytic bias terms.
"""
import sys

if "/opt/trn_rl_repo" not in sys.path:
    sys.path.insert(0, "/opt/trn_rl_repo")

import os
import numpy as np
import concourse.bass as bass
import concourse.tile as tile
from concourse import mybir
from concourse.bass_utils import run_bass_kernel_spmd

B = 2048
W = 512
E = 16
TOPK = 2
INTER = 1024
SH = 2048
OUT = 128
NCORES = 8
EPC = E // NCORES          # experts per core = 2
SHS = SH // NCORES         # shared-expert inter slice per core = 256
KW = W // 128              # k-tiles over W = 4
MI = INTER // 128          # m-tiles over INTER = 8
MS = SHS // 128            # m-tiles over shared slice = 2
MBLK = 2 * KW * 128 + OUT        # columns per m-tile weight block: w1|w3|w2ot
GCOLS = 8 * MBLK                 # columns per expert (8 m-tile blocks)
F32 = mybir.dt.float32
F16 = mybir.dt.float16
DT = F16                   # device datapath dtype for matmul operands
NPDT = np.float16

# set by test.py to collect a profile; results stashed in LAST_RESULTS
TRACE = False
TRACE_KW = {}
LAST_RESULTS = None


def _legalize_waits(nc):
    """This container's walrus accepts at most 1 sync wait per instruction
    (2 for EventSemaphore).  Hoist excess waits emitted by the Tile
    scheduler into standalone EventSemaphore instructions."""
    for fn in nc.m.functions:
        for blk in fn.blocks:
            out = []
            changed = False
            for inst in blk.instructions:
                si = getattr(inst, "sync_info", None)
                waits = list(si.on_wait) if si is not None and si.on_wait else []
                cap = 2 if isinstance(inst, mybir.InstEventSemaphore) else 1
                if len(waits) > cap:
                    extra, keep = waits[:-cap], waits[-cap:]
                    for i in range(0, len(extra), 2):
                        out.append(mybir.InstEventSemaphore(
                            name=nc.get_next_instruction_name(),
                            engine=inst.engine,
                            ins=[], outs=[],
                            sync_info=mybir.SyncInfo(
                                on_wait=list(extra[i:i + 2]), on_update=[]),
                        ))
                    si.on_wait = keep
                    changed = True
                out.append(inst)
            if changed:
                blk.instructions = out
    return nc


def _build_nc(cap0, cap1, legalize=True):
    """SPMD Bass program for per-slot token capacities (cap0, cap1)."""
    nc = bass.Bass("TRN2", target_bir_lowering=False, debug=False)
    caps = (cap0, cap1)

    def din(name, f, dt=DT):
        return nc.dram_tensor(name, [128, f], dt, kind="ExternalInput").ap()

    xga = din("xga", KW * cap0)     # gathered tokens, slot A, k-major
    xgb = din("xgb", KW * cap1)
    wga = din("wga", GCOLS)         # slot A weights: 8 m-blocks [w1|w3|w2ot]
    wgb = din("wgb", GCOLS)
    swp = din("swp", MS * MBLK)     # shared weights: 2 m-blocks
    xt = din("xt", KW * B)          # x.T in 4 chunk-major blocks of [128, KW*512]
    bias = din("bias", 4 * MI + 2 * MS, F32)

    yr = nc.dram_tensor("yr", [128, cap0 + cap1], DT, kind="ExternalOutput").ap()
    zt = nc.dram_tensor("zt", [128, B], DT, kind="ExternalOutput").ap()

    LR = mybir.ActivationFunctionType.Lrelu

    with tile.TileContext(nc) as tc:
        with tc.tile_pool(name="wts", bufs=1) as wts, \
             tc.tile_pool(name="work", bufs=3) as work, \
             tc.tile_pool(name="hts", bufs=1) as hts, \
             tc.tile_pool(name="outs", bufs=2) as outs, \
             tc.tile_pool(name="ps", bufs=2, space="PSUM") as ps:

            # ---- PE warmup tile (zeroed by Pool engine; no DMA dependency)
            warm = wts.tile([128, 512], DT, tag="warm")
            nc.gpsimd.memset(warm[:], 0.0)

            # ---- input DMAs, consumption-ordered on the two HWDGE queues
            bias_t = wts.tile([128, bias.shape[1]], F32, tag="bias")
            xga_t = wts.tile([128, KW * cap0], DT, tag="xga")
            xgb_t = wts.tile([128, KW * cap1], DT, tag="xgb")
            swp_t = wts.tile([128, MS * MBLK], DT, tag="swp")
            xt_ts = [wts.tile([128, KW * 512], DT, tag=f"xt{c}", name=f"xt{c}")
                     for c in range(4)]
            wga_t = wts.tile([128, GCOLS], DT, tag="wga")
            wgb_t = wts.tile([128, GCOLS], DT, tag="wgb")

            # ACT queue: bias + shared stream first (most PE-work per byte),
            # xgb mid-stream for the B slot.
            nc.scalar.dma_start(bias_t[:], bias[:])
            nc.scalar.dma_start(swp_t[:], swp[:])
            nc.scalar.dma_start(xt_ts[0][:], xt[:, 0:KW * 512])
            nc.scalar.dma_start(xgb_t[:], xgb[:])
            for c in range(1, 4):
                nc.scalar.dma_start(xt_ts[c][:], xt[:, c * KW * 512:(c + 1) * KW * 512])
            # SP queue: slot-A m0 weights, then xga k-chunks (so A-m0's
            # k-loop can start before the full gather lands), then the rest.
            nc.sync.dma_start(wga_t[:, :MBLK], wga[:, :MBLK])
            for k in range(KW):
                nc.sync.dma_start(xga_t[:, k * cap0:(k + 1) * cap0],
                                  xga[:, k * cap0:(k + 1) * cap0])
            nc.sync.dma_start(wga_t[:, MBLK:2 * MBLK], wga[:, MBLK:2 * MBLK])
            for g in range(1, 4):
                nc.sync.dma_start(wga_t[:, 2 * g * MBLK:2 * (g + 1) * MBLK],
                                  wga[:, 2 * g * MBLK:2 * (g + 1) * MBLK])
            for g in range(4):
                nc.sync.dma_start(wgb_t[:, 2 * g * MBLK:2 * (g + 1) * MBLK],
                                  wgb[:, 2 * g * MBLK:2 * (g + 1) * MBLK])

            # ---- PE warmup: one accumulation group of cold N=512 matmuls
            # (~4.5us) keeps the PE busy through the DMA ramp and flips the
            # HAM clock gate to 8/8 before real work arrives.
            NWARM = 10
            pw = ps.tile([128, 512], F32, tag="warm", bufs=1)
            for i in range(NWARM):
                nc.tensor.matmul(pw[:], warm[:, 0:128], warm[:],
                                 start=(i == 0), stop=(i == NWARM - 1))

            def b_ap(col):  # [128,1] per-partition bias column
                return bias_t[:, col:col + 1]

            LAG = 2

            def expert_slot(slot):
                """Generator: one step per m-tile, stage-2 trails by LAG."""
                cap = caps[slot]
                xg_t = (xga_t, xgb_t)[slot]
                wg_t = (wga_t, wgb_t)[slot]
                boff = slot * 2 * MI
                py = ps.tile([128, cap], F32, tag="py", bufs=1)
                pend = []

                def stage2(m, ht):
                    lhs = wg_t[:, m * MBLK + KW * 256:m * MBLK + KW * 256 + OUT]
                    nc.tensor.matmul(py[:], lhs, ht[:],
                                     start=(m == 0), stop=(m == MI - 1))

                for m in range(MI):
                    mb = m * MBLK
                    p1 = ps.tile([128, cap], F32, tag="p1", bufs=2)
                    p3 = ps.tile([128, cap], F32, tag="p3", bufs=2)
                    for k in range(KW):
                        lhs = wg_t[:, mb + k * 128:mb + (k + 1) * 128]
                        rhs = xg_t[:, k * cap:(k + 1) * cap]
                        nc.tensor.matmul(p1[:], lhs, rhs, start=(k == 0), stop=(k == KW - 1))
                    for k in range(KW):
                        lhs = wg_t[:, mb + KW * 128 + k * 128:mb + KW * 128 + (k + 1) * 128]
                        rhs = xg_t[:, k * cap:(k + 1) * cap]
                        nc.tensor.matmul(p3[:], lhs, rhs, start=(k == 0), stop=(k == KW - 1))
                    a = work.tile([128, cap], DT, tag="act_a")
                    nc.scalar.activation(a[:], p1[:], LR, bias=b_ap(boff + m), alpha=0.01)
                    t3 = work.tile([128, cap], DT, tag="act_b")
                    nc.vector.tensor_scalar_add(t3[:], p3[:], b_ap(boff + MI + m))
                    ht = hts.tile([128, cap], DT, tag="ht", bufs=LAG + 3)
                    nc.gpsimd.tensor_mul(ht[:], a[:], t3[:])
                    pend.append((m, ht))
                    if len(pend) > LAG:
                        stage2(*pend.pop(0))
                    if m != MI - 1:
                        yield
                for args in pend:
                    stage2(*args)
                yo = outs.tile([128, cap], DT, tag="yo")
                nc.vector.tensor_copy(yo[:], py[:])
                off = 0 if slot == 0 else cap0
                nc.sync.dma_start(yr[:, off:off + cap], yo[:])
                yield

            def shared_steps():
                """Generator: one step per (chunk, m-tile); stage-2 flush of
                chunk c happens during chunk c+1 (full-chunk lag)."""
                pend = []

                def flush(pz, c, hs_list):
                    for m, hs in hs_list:
                        lhs = swp_t[:, m * MBLK + KW * 256:m * MBLK + KW * 256 + OUT]
                        nc.tensor.matmul(pz[:], lhs, hs[:], start=(m == 0), stop=(m == MS - 1))
                    zo = outs.tile([128, 512], DT, tag="zo")
                    nc.vector.tensor_copy(zo[:], pz[:])
                    nc.sync.dma_start(zt[:, c * 512:(c + 1) * 512], zo[:])

                for c in range(4):
                    pz = ps.tile([128, 512], F32, tag="pz", bufs=1)
                    hs_list = []
                    for m in range(MS):
                        mb = m * MBLK
                        p1 = ps.tile([128, 512], F32, tag="p1", bufs=2)
                        p3 = ps.tile([128, 512], F32, tag="p3", bufs=2)
                        for k in range(KW):
                            lhs = swp_t[:, mb + k * 128:mb + (k + 1) * 128]
                            rhs = xt_ts[c][:, k * 512:(k + 1) * 512]
                            nc.tensor.matmul(p1[:], lhs, rhs, start=(k == 0), stop=(k == KW - 1))
                        for k in range(KW):
                            lhs = swp_t[:, mb + KW * 128 + k * 128:mb + KW * 128 + (k + 1) * 128]
                            rhs = xt_ts[c][:, k * 512:(k + 1) * 512]
                            nc.tensor.matmul(p3[:], lhs, rhs, start=(k == 0), stop=(k == KW - 1))
                        a = work.tile([128, 512], DT, tag="act_a")
                        nc.scalar.activation(a[:], p1[:], LR,
                                             bias=b_ap(4 * MI + m), alpha=0.01)
                        t3 = work.tile([128, 512], DT, tag="act_b")
                        nc.vector.tensor_scalar_add(t3[:], p3[:], b_ap(4 * MI + MS + m))
                        hs = hts.tile([128, 512], DT, tag="hs", bufs=MS + 2)
                        nc.gpsimd.tensor_mul(hs[:], a[:], t3[:])
                        hs_list.append((m, hs))
                        if not (c == 3 and m == MS - 1):
                            yield
                    pend.append((pz, c, hs_list))
                    if len(pend) > 1:
                        flush(*pend.pop(0))
                for args in pend:
                    flush(*args)
                yield

            # ---- interleave: A/B = expert slot m-tile, S = shared m-tile.
            # Default tuned to the DMA arrival schedule (SP ~0.58MB/group,
            # ACT xga+swp+xt chunks); override via K_PATTERN for experiments.
            pattern = os.environ.get(
                "K_PATTERN",
                "AA SS AA SS AA SS AA BB SS BB BB BB")
            gens = {"A": expert_slot(0), "B": expert_slot(1), "S": shared_steps()}
            for ch in pattern:
                if ch == " ":
                    continue
                g = gens.get(ch)
                if g is None:
                    continue
                try:
                    next(g)
                except StopIteration:
                    gens[ch] = None
            for g in gens.values():
                if g is None:
                    continue
                for _ in g:
                    pass

    if legalize:
        _legalize_waits(nc)
    return nc


_NC_CACHE = {}


def _kblocks(mat, nk):
    """[nk*128, F] -> [128, nk*F], col block k = mat[128k:128(k+1), :]."""
    f = mat.shape[1]
    return mat.reshape(nk, 128, f).transpose(1, 0, 2).reshape(128, -1)


def _pack_mblocks(w1e, w3e, w2oe, nm):
    """Consumption-ordered weight pack: nm m-blocks of
    [w1 k-blocks (KW*128) | w3 k-blocks | w2ot (OUT)] = MBLK cols each."""
    # w1e/w3e: [nm*128, W];  w2oe: [nm*128, OUT]
    w1b = w1e.T.reshape(KW, 128, nm, 128)   # [k, kr, m, mc]
    w3b = w3e.T.reshape(KW, 128, nm, 128)
    w2b = w2oe.reshape(nm, 128, OUT)        # [m, mr, OUT]
    cols = []
    for m in range(nm):
        for k in range(KW):
            cols.append(w1b[k, :, m, :])
        for k in range(KW):
            cols.append(w3b[k, :, m, :])
        cols.append(w2b[m])
    return np.concatenate(cols, axis=1).astype(NPDT)


def prepare(x, task_id, gate_w, w1, b1, w2, b2, w3, b3,
            sw1, sb1, sw2, sb2, sw3, sb3, ow, ob):
    """Host-side routing + packing."""
    x = np.asarray(x, np.float32)
    f32 = lambda a: np.asarray(a, np.float32)
    gate_w, w1, b1, w2, b2, w3, b3 = map(f32, (gate_w, w1, b1, w2, b2, w3, b3))
    sw1, sb1, sw2, sb2, sw3, sb3, ow, ob = map(f32, (sw1, sb1, sw2, sb2, sw3, sb3, ow, ob))

    # ---- host gate: softmax + top-2 ----
    logits = x @ gate_w.T
    logits -= logits.max(axis=1, keepdims=True)
    ex = np.exp(logits)
    scores = ex / ex.sum(axis=1, keepdims=True)
    order = np.argsort(-scores, axis=1, kind="stable")[:, :TOPK]

    tok_lists = [np.nonzero((order == e).any(axis=1))[0] for e in range(E)]
    counts = np.array([len(t) for t in tok_lists])
    rank = np.argsort(-counts, kind="stable")
    slotA = [int(rank[i]) for i in range(NCORES)]          # big experts
    slotB = [int(rank[E - 1 - i]) for i in range(NCORES)]  # small experts
    r16 = lambda n: max(64, -(-n // 16) * 16)
    cap0 = r16(max(counts[e] for e in slotA))
    cap1 = r16(max(counts[e] for e in slotB))

    key = (cap0, cap1)
    if key not in _NC_CACHE:
        _NC_CACHE[key] = _build_nc(cap0, cap1)
    nc = _NC_CACHE[key]

    # ---- shared-expert packing (same for every core except slice) ----
    xtp = x.T.reshape(KW, 128, 4, 512).transpose(1, 2, 0, 3).reshape(128, -1)
    xtp = np.ascontiguousarray(xtp).astype(NPDT)   # chunk-major, k inside

    in_maps = []
    for c in range(NCORES):
        eA, eB = slotA[c], slotB[c]
        s = slice(c * SHS, (c + 1) * SHS)

        def gather(e, cap):
            toks = tok_lists[e]
            xg = np.zeros((W, cap), np.float32)
            xg[:, :len(toks)] = x[toks].T
            return _kblocks(xg, KW).astype(NPDT)

        swp = _pack_mblocks(sw1[s], sw3[s], sw2[:, s].T @ ow.T, MS)

        bias_cols = [b1[eA].reshape(MI, 128).T, b3[eA].reshape(MI, 128).T,
                     b1[eB].reshape(MI, 128).T, b3[eB].reshape(MI, 128).T,
                     sb1[s].reshape(MS, 128).T, sb3[s].reshape(MS, 128).T]

        in_maps.append({
            "xga": gather(eA, cap0),
            "xgb": gather(eB, cap1),
            "wga": _pack_mblocks(w1[eA], w3[eA], w2[eA].T @ ow.T, MI),
            "wgb": _pack_mblocks(w1[eB], w3[eB], w2[eB].T @ ow.T, MI),
            "swp": swp,
            "xt": xtp,
            "bias": np.ascontiguousarray(np.concatenate(bias_cols, axis=1)),
        })

    combine_w = np.zeros((B, E), np.float32)
    rows = np.arange(B)
    combine_w[rows[:, None], order] = np.take_along_axis(scores, order, axis=1)
    base = combine_w @ (b2 @ ow.T) + sb2 @ ow.T + ob

    return dict(nc=nc, in_maps=in_maps, cap0=cap0, cap1=cap1,
                slotA=slotA, slotB=slotB, tok_lists=tok_lists,
                combine_w=combine_w, base=base)


def combine(p, results):
    """Combine per-core device partials into the full [B, OUT] output."""
    cap0, tok_lists, combine_w = p["cap0"], p["tok_lists"], p["combine_w"]
    out = p["base"].astype(np.float32).copy()
    for c in range(NCORES):
        r = results[c]
        out += r["zt"].astype(np.float32).T
        for slot, e in ((0, p["slotA"][c]), (1, p["slotB"][c])):
            toks = tok_lists[e]
            off = 0 if slot == 0 else cap0
            yre = r["yr"][:, off:off + len(toks)].astype(np.float32)
            out[toks] += combine_w[toks, e][:, None] * yre.T
    return out


def kernel(x, task_id, gate_w, w1, b1, w2, b2, w3, b3,
           sw1, sb1, sw2, sb2, sw3, sb3, ow, ob):
    global LAST_RESULTS
    p = prepare(x, task_id, gate_w, w1, b1, w2, b2, w3, b3,
                sw1, sb1, sw2, sb2, sw3, sb3, ow, ob)
    res = run_bass_kernel_spmd(
        p["nc"], p["in_maps"], core_ids=list(range(NCORES)),
        trace=TRACE, **TRACE_KW)
    LAST_RESULTS = res
    return combine(p, res.results)
